# revision 1
# baseline (speedup 1.0000x reference)
"""Trainium2 Bass kernel for nn_ExperimentModel (embed -> LN -> S4D -> mean-pool -> linear).

Math: the output is pooled over L, and mean-pooling commutes with the causal
conv, so  pooled[b,m] = (1/L) * sum_l u[b,l,m] * W[m,l]  with
  W[m,l] = CK[m, L-1-l] + D[m],   CK[m,t] = sum_n C[m,n] (1-q_n^{t+1})/(1-q_n),
  q_n = exp(dt * A_n)  (dt is a scalar: log_dt is uniform by construction).
W[m,l] = WcD[m] - rho[m, L-l],  rho[m,s] = sum_n g[m,n] q_n^s,  g = C/(1-q),
and rho is negligible for s > LTAIL (q_n <= e^-0.01): for l < L-LTAIL the
weight is the per-channel constant WcD[m].

u = LN(emb[x]) is a gather from a pre-normalized 2000-row bf16 table kept in
SBUF, fetched with SBUF-source transposed dma_gather into [m=128 part, l]
layout (chunked at <=896 idxs/call: the SWDGE ring holds 64 descriptors/ring
and a transposed gather needs n/16+2). Bulk chunks reduce via tensor_scalar
(scalar1 = WcD/L per partition) with accum_out; only the last LTAIL positions
use a materialized tail weight via scalar_tensor_tensor. Batch-parallel over
8 cores (4 batches each).
"""

import numpy as np
from contextlib import ExitStack

import concourse.bass as bass
import concourse.bacc as bacc
import concourse.tile as tile
from concourse import mybir

B, L, V, M, N = 32, 4096, 2000, 128, 64
LTAIL = 1024
NCORES = 8
BPC = B // NCORES
LN_EPS = 1e-5
VPAD = 2048          # vocab rows padded to 16 ranks of 128
NRANK = VPAD // 128

# gather chunking: [896 x4, 512]; chunk 3 straddles the L-LTAIL boundary
GCHUNKS = [(0, 896), (896, 896), (1792, 896), (2688, 896), (3584, 512)]

# params column map (f32 [128, PRM_F]); ramp is a separate [64, LTAIL] input
ALOG0 = 0                  # [0:64, 0]         A_log
DCOL0 = 1                  # [:, 1]            D
LOGDT0 = 2                 # [:, 2]            log_dt (as column over m)
WCLS0 = 3                  # [:, 3:5]          W_cls^T
CT0 = 5                    # [0:64, 5:133]     C_re^T
LNW0 = 133                 # [0:1, 133:261]    ln_w row
LNB0 = 261                 # [0:1, 261:389]    ln_b row
BCLS0 = 389                # [0:1, 389:391]    b_cls
PRM_F = 448

f32 = mybir.dt.float32
bf16 = mybir.dt.bfloat16
i16 = mybir.dt.int16
AF = mybir.ActivationFunctionType
OP = mybir.AluOpType


def declare_io(nc):
    emb_d = nc.dram_tensor("emb_l", [128, NRANK * 128], f32, kind="ExternalInput")
    idx_d = nc.dram_tensor("idx_l", [128, BPC * L // 16], i16, kind="ExternalInput")
    blob_d = nc.dram_tensor("blob_l", [128, PRM_F], f32, kind="ExternalInput")
    ramp_d = nc.dram_tensor("ramp_l", [64, LTAIL], f32, kind="ExternalInput")
    out_d = nc.dram_tensor("out", [BPC, 2], f32, kind="ExternalOutput")
    return emb_d, idx_d, blob_d, ramp_d, out_d


ITER_COUNT_DEBUG = [False]


def emit_body(nc, tc, ctx, io, ln_affine):
    emb_d, idx_d, blob_d, ramp_d, out_d = io
    if True:
        if True:
            singles = ctx.enter_context(tc.tile_pool(name="singles", bufs=1))
            work = ctx.enter_context(tc.tile_pool(name="work", bufs=2))
            small = ctx.enter_context(tc.tile_pool(name="small", bufs=1))
            psum = ctx.enter_context(tc.tile_pool(name="psum", bufs=1, space="PSUM"))

            # ---- input loads; params first, emb split 4-way for early LN start
            blob = singles.tile([128, PRM_F], f32)
            nc.sync.dma_start(out=blob, in_=blob_d[:])
            idx_sb = singles.tile([128, BPC * L // 16], i16)
            nc.sync.dma_start(out=idx_sb, in_=idx_d[:])
            emb_sb = singles.tile([128, NRANK, 128], f32)
            emb_v = emb_d[:].rearrange("p (r m) -> p r m", m=128)
            for h in range(2):
                nc.sync.dma_start(out=emb_sb[:, 8 * h:8 * h + 8, :],
                                  in_=emb_v[:, 8 * h:8 * h + 8, :])
            ramp_sb = singles.tile([64, LTAIL], f32)
            nc.sync.dma_start(out=ramp_sb, in_=ramp_d[:])

            # ---- S4D weight construction (tiny ops + 3 small matmuls)
            dt_col = small.tile([128, 1], f32)
            nc.scalar.activation(dt_col, blob[:, LOGDT0:LOGDT0 + 1], AF.Exp)
            expa = small.tile([64, 1], f32)
            nc.scalar.activation(expa, blob[0:64, ALOG0:ALOG0 + 1], AF.Exp)
            ones_1x64 = small.tile([1, 64], f32)
            nc.vector.memset(ones_1x64, 1.0)
            dt0_ps = psum.tile([64, 1], f32)
            nc.tensor.matmul(dt0_ps, lhsT=ones_1x64, rhs=dt_col[0:1, :], start=True, stop=True)
            c_col = small.tile([64, 1], f32)  # c_n = -exp(A_log_n)*dt  (negative)
            nc.vector.scalar_tensor_tensor(
                out=c_col, in0=expa, scalar=-1.0, in1=dt0_ps,
                op0=OP.mult, op1=OP.mult)
            q_col = small.tile([64, 1], f32)
            nc.scalar.activation(q_col, c_col, AF.Exp)
            one_col = small.tile([64, 1], f32)
            nc.vector.memset(one_col, 1.0)
            omq = small.tile([64, 1], f32)
            nc.vector.tensor_sub(omq, one_col, q_col)
            wrec = small.tile([64, 1], f32)
            nc.vector.reciprocal(wrec, omq)
            g_sb = singles.tile([64, 128], f32)  # g^T[n,m] = C^T[n,m] / (1 - q_n)
            nc.vector.tensor_scalar_mul(g_sb, blob[0:64, CT0:CT0 + 128], scalar1=wrec)
            p_sb = singles.tile([64, LTAIL], f32)  # q_n^(LTAIL - j)
            nc.scalar.activation(p_sb, ramp_sb, AF.Exp, scale=c_col)
            rho_ps = []
            for h in range(2):
                rp = psum.tile([128, 512], f32, tag=f"rho{h}")
                nc.tensor.matmul(rp, lhsT=g_sb, rhs=p_sb[:, h * 512:(h + 1) * 512],
                                 start=True, stop=True)
                rho_ps.append(rp)
            kt_ps = psum.tile([128, 1], f32)
            nc.tensor.matmul(kt_ps, lhsT=g_sb, rhs=one_col, start=True, stop=True)
            wcd = small.tile([128, 1], f32)  # (Ktot + D) / L
            nc.vector.tensor_add(wcd, kt_ps, blob[:, DCOL0:DCOL0 + 1])
            nc.vector.tensor_scalar_mul(wcd, wcd, scalar1=1.0 / L)

            # ---- tail weight W^T[m, l] for l in [L-LTAIL, L): (WcD - rho), unscaled
            w_t = singles.tile([128, LTAIL], bf16)
            nc.vector.memset(w_t, 1.0)
            wcd_raw = small.tile([128, 1], f32)
            nc.vector.tensor_add(wcd_raw, kt_ps, blob[:, DCOL0:DCOL0 + 1])
            nc.vector.tensor_scalar_mul(w_t, w_t, scalar1=wcd_raw)
            for h in range(2):
                sl = slice(h * 512, (h + 1) * 512)
                nc.vector.scalar_tensor_tensor(
                    out=w_t[:, sl], in0=rho_ps[h], scalar=-1.0, in1=w_t[:, sl],
                    op0=OP.mult, op1=OP.add)

            # ---- LN of the vocab table -> ENORM bf16 [v%128, v//128, m]
            # stats on DVE (bn_stats/aggr); rstd = exp(-0.5*ln(var+eps)) and the
            # normalize itself on ACT (Identity with per-partition scale/bias),
            # all within the natural_log_exp table set.
            enorm = singles.tile([128, NRANK, 128], bf16)
            # per-quarter stats (pipelines with the emb DMA quarters)
            mean = small.tile([128, NRANK], f32)
            sume = small.tile([128, NRANK], f32)
            sumsq = small.tile([128, NRANK], f32)
            for hq in range(2):
                qs = slice(8 * hq, 8 * hq + 8)
                nc.vector.tensor_reduce(out=sume[:, qs], in_=emb_sb[:, qs, :],
                                        axis=mybir.AxisListType.X, op=OP.add)
                sqq = work.tile([128, 8, 128], f32, tag="sq")
                nc.scalar.square(sqq, emb_sb[:, qs, :])
                nc.vector.tensor_reduce(out=sumsq[:, qs], in_=sqq,
                                        axis=mybir.AxisListType.X, op=OP.add)
            nc.vector.tensor_scalar_mul(mean, sume, scalar1=1.0 / 128.0)
            meansq = small.tile([128, NRANK], f32)
            nc.vector.tensor_mul(meansq, mean, mean)
            vpe = small.tile([128, NRANK], f32)  # var + eps
            nc.vector.scalar_tensor_tensor(
                out=vpe, in0=sumsq, scalar=1.0 / 128.0, in1=meansq,
                op0=OP.mult, op1=OP.subtract)
            nc.vector.tensor_single_scalar(vpe, vpe, LN_EPS, OP.add)
            # rstd = rsqrt(var+eps): Quake seed + 2 Newton steps, all on DVE
            # (no sqrt ACT table set -> the whole kernel uses one table load)
            shi = small.tile([128, NRANK], mybir.dt.uint32)
            nc.vector.tensor_single_scalar(
                shi, vpe.bitcast(mybir.dt.uint32), 1, OP.logical_shift_right)
            y = small.tile([128, NRANK], mybir.dt.int32)
            nc.vector.tensor_scalar(
                out=y, in0=shi, scalar1=-1.0, scalar2=float(0x5F3759DF),
                op0=OP.mult, op1=OP.add)  # int32 out: value C - (v>>1)
            rstd_all = small.tile([128, NRANK], f32)
            yf = y.bitcast(f32)
            t1 = small.tile([128, NRANK], f32)
            cur = yf
            for it in range(2):
                nc.vector.tensor_mul(t1, cur, cur)
                nc.vector.scalar_tensor_tensor(
                    out=t1, in0=t1, scalar=-0.5, in1=vpe, op0=OP.mult, op1=OP.mult)
                nc.vector.scalar_tensor_tensor(
                    out=rstd_all, in0=t1, scalar=1.5, in1=cur, op0=OP.add, op1=OP.mult)
                cur = rstd_all
            negmurs = small.tile([128, NRANK], f32)
            nc.vector.scalar_tensor_tensor(
                out=negmurs, in0=mean, scalar=-1.0, in1=rstd_all,
                op0=OP.mult, op1=OP.mult)
            if ln_affine:
                ones_1x128 = small.tile([1, 128], f32)
                nc.vector.memset(ones_1x128, 1.0)
                lnw_ps = psum.tile([128, 128], f32)
                nc.tensor.matmul(lnw_ps, lhsT=ones_1x128, rhs=blob[0:1, LNW0:LNW0 + 128],
                                 start=True, stop=True)
                lnb_ps = psum.tile([128, 128], f32)
                nc.tensor.matmul(lnb_ps, lhsT=ones_1x128, rhs=blob[0:1, LNB0:LNB0 + 128],
                                 start=True, stop=True)
                lnw_bc = singles.tile([128, 128], f32)
                nc.scalar.copy(lnw_bc, lnw_ps)
                lnb_bc = singles.tile([128, 128], f32)
                nc.scalar.copy(lnb_bc, lnb_ps)
            for s in range(NRANK):
                if ln_affine:
                    tmp = small.tile([128, 128], f32, tag="lntmp")
                    nc.scalar.activation(tmp, emb_sb[:, s, :], AF.Identity,
                                         bias=negmurs[:, s:s + 1],
                                         scale=rstd_all[:, s:s + 1])
                    nc.vector.tensor_mul(tmp, tmp, lnw_bc)
                    nc.vector.tensor_add(enorm[:, s, :], tmp, lnb_bc)
                elif s % 2 == 0:
                    nc.scalar.activation(enorm[:, s, :], emb_sb[:, s, :], AF.Identity,
                                         bias=negmurs[:, s:s + 1],
                                         scale=rstd_all[:, s:s + 1])
                else:
                    nc.vector.tensor_scalar(
                        out=enorm[:, s, :], in0=emb_sb[:, s, :],
                        scalar1=mean[:, s:s + 1], scalar2=rstd_all[:, s:s + 1],
                        op0=OP.subtract, op1=OP.mult)

            # ---- per-batch chunked gather + fused weighted reduce
            enorm_flat = enorm.rearrange("p r m -> p (r m)")
            tail0 = L - LTAIL
            pooled = small.tile([128, BPC], f32)
            for b in range(BPC):
                u_b = work.tile([128, L], bf16, tag="u")
                pc = work.tile([128, 6], f32, tag="pc")
                acc = 0
                for ci, (c0, n) in enumerate(GCHUNKS):
                    nc.gpsimd.dma_gather(
                        out_ap=u_b[:, c0:c0 + n].rearrange("p (c l) -> p c l", c=1),
                        in_ap=enorm_flat,
                        idxs_ap=idx_sb[:, (b * L + c0) // 16:(b * L + c0 + n) // 16],
                        num_idxs=n,
                        num_idxs_reg=n,
                        elem_size=128,
                        transpose=True,
                        sbuf_tokens_per_rank=128,
                        sbuf_free_dim_per_rank=256,
                        queue_num=(b * len(GCHUNKS) + ci) % 4,
                    )
                    spans = []
                    if c0 < tail0:
                        spans.append((c0, min(c0 + n, tail0), False))
                    if c0 + n > tail0:
                        spans.append((max(c0, tail0), c0 + n, True))
                    for (s0, s1, is_tail) in spans:
                        if is_tail:
                            prod = work.tile([128, LTAIL], bf16, tag="prod")
                            nc.vector.scalar_tensor_tensor(
                                out=prod[:, :s1 - s0],
                                in0=u_b[:, s0:s1],
                                scalar=1.0 / L,
                                in1=w_t[:, s0 - tail0:s1 - tail0],
                                op0=OP.mult,
                                op1=OP.mult,
                                accum_out=pc[:, acc:acc + 1],
                            )
                        else:
                            prod = work.tile([128, 896], bf16, tag="prodb")
                            nc.vector.tensor_scalar(
                                out=prod[:, :s1 - s0],
                                in0=u_b[:, s0:s1],
                                scalar1=wcd,
                                scalar2=None,
                                op0=OP.mult,
                                op1=OP.add,
                                accum_out=pc[:, acc:acc + 1],
                            )
                        acc += 1
                assert acc == 6
                nc.vector.tensor_reduce(
                    out=pooled[:, b:b + 1], in_=pc, axis=mybir.AxisListType.X,
                    op=OP.add)

            # ---- classifier: logits = pooled^T @ Wcls^T + b_cls
            ones_1xb = small.tile([1, BPC], f32)
            nc.vector.memset(ones_1xb, 1.0)
            logits_ps = psum.tile([BPC, 2], f32)
            nc.tensor.matmul(logits_ps, lhsT=pooled, rhs=blob[:, WCLS0:WCLS0 + 2],
                             start=True, stop=False)
            nc.tensor.matmul(logits_ps, lhsT=ones_1xb, rhs=blob[0:1, BCLS0:BCLS0 + 2],
                             start=False, stop=True)
            out_sb = small.tile([BPC, 2], f32)
            nc.vector.tensor_copy(out_sb, logits_ps)
            if ITER_COUNT_DEBUG[0]:
                cnt = small.tile([BPC, 2], f32, tag="itercnt")
                nc.vector.tensor_single_scalar(cnt, cnt, 1.0, OP.add)
                nc.vector.tensor_copy(out_sb, cnt)
            nc.sync.dma_start(out=out_d[:], in_=out_sb)


def build_program(ln_affine: bool, repeat: int = 1):
    nc = bacc.Bacc("TRN2", target_bir_lowering=False, debug=False,
                   num_swdge_queues=4)
    io = declare_io(nc)
    with tile.TileContext(nc) as tc:
        with ExitStack() as ctx:
            if repeat == 1:
                emit_body(nc, tc, ctx, io, ln_affine)
            else:
                with tc.For_i(0, repeat, 1):
                    emit_body(nc, tc, ctx, io, ln_affine)
    nc.compile()
    return nc


_PROG_CACHE = {}


def _get_prog(ln_affine: bool):
    if ln_affine not in _PROG_CACHE:
        _PROG_CACHE[ln_affine] = build_program(ln_affine)
    return _PROG_CACHE[ln_affine]


def host_prep(x, emb, ln_w, ln_b, A_log, D, C_re, log_dt, W_cls, b_cls):
    """Pure data marshaling: reshape/transposes/pads, no arithmetic on values."""
    x = np.asarray(x)
    assert x.shape == (B, L)
    emb = np.asarray(emb, dtype=np.float32)
    assert np.allclose(log_dt, log_dt[0]), "factorized S4D path needs scalar dt"

    emb_pad = np.zeros((VPAD, M), dtype=np.float32)
    emb_pad[:V] = emb
    emb_l = np.ascontiguousarray(
        emb_pad.reshape(NRANK, 128, M).transpose(1, 0, 2).reshape(128, NRANK * M))

    ramp_l = np.ascontiguousarray(
        np.tile((LTAIL - np.arange(LTAIL, dtype=np.float32))[None, :], (64, 1)))
    blob = np.zeros((128, PRM_F), dtype=np.float32)
    blob[0:64, ALOG0] = np.asarray(A_log, dtype=np.float32)
    blob[:, DCOL0] = np.asarray(D, dtype=np.float32)
    blob[:, LOGDT0] = np.asarray(log_dt, dtype=np.float32)
    blob[:, WCLS0:WCLS0 + 2] = np.asarray(W_cls, dtype=np.float32).T
    blob[0, LNW0:LNW0 + 128] = np.asarray(ln_w, dtype=np.float32)
    blob[0, LNB0:LNB0 + 128] = np.asarray(ln_b, dtype=np.float32)
    blob[0, BCLS0:BCLS0 + 2] = np.asarray(b_cls, dtype=np.float32)
    blob[0:64, CT0:CT0 + 128] = np.asarray(C_re, dtype=np.float32).T

    ln_affine = not (np.all(np.asarray(ln_w) == 1.0) and np.all(np.asarray(ln_b) == 0.0))

    in_maps = []
    for k in range(NCORES):
        xc = x[k * BPC:(k + 1) * BPC].astype(np.int16).reshape(-1)   # [BPC*L]
        idx_l = np.ascontiguousarray(
            np.tile(xc.reshape(-1, 16).T, (8, 1)))                   # [128, BPC*L/16]
        in_maps.append({"emb_l": emb_l, "idx_l": idx_l, "blob_l": blob,
                        "ramp_l": ramp_l})
    return in_maps, ln_affine


def kernel(**inputs):
    from concourse.bass_utils import run_bass_kernel_spmd

    in_maps, ln_affine = host_prep(**inputs)
    nc = _get_prog(ln_affine)
    res = run_bass_kernel_spmd(nc, in_maps, core_ids=list(range(NCORES)))
    out = np.concatenate([res.results[k]["out"] for k in range(NCORES)], axis=0)
    return out.astype(np.float32)



# revision 7
# speedup vs baseline: 1.0694x; 1.0694x over previous
"""Trainium2 Bass kernel for nn_ExperimentModel (embed -> LN -> S4D -> mean-pool -> linear).

Math: the output is pooled over L, and mean-pooling commutes with the causal
conv, so  pooled[b,m] = (1/L) * sum_l u[b,l,m] * W[m,l]  with
  W[m,l] = CK[m, L-1-l] + D[m],   CK[m,t] = sum_n C[m,n] (1-q_n^{t+1})/(1-q_n),
  q_n = exp(dt * A_n)  (dt is a scalar: log_dt is uniform by construction).
W[m,l] = WcD[m] - rho[m, L-l],  rho[m,s] = sum_n g[m,n] q_n^s,  g = C/(1-q),
and rho is negligible for s > LTAIL (q_n <= e^-0.01): for l < L-LTAIL the
weight is the per-channel constant WcD[m].

u = LN(emb[x]) is a gather from a pre-normalized 2000-row bf16 table kept in
SBUF, fetched with SBUF-source transposed dma_gather into [m=128 part, l]
layout. Schedule (vs the first version): inputs are minimal (idx loaded once
into 16 partitions and replicated on-chip; the exp ramp is an on-chip iota;
params are three small tensors), the vocab-table LN is pipelined per emb
quarter via bn_stats/bn_aggr, a warmup dma_gather absorbs the SWDGE
ring/library init while the table is still loading, and all BPC*L positions
live in one contiguous u tile so the gather chunks (<=896 idxs: the SWDGE
ring holds 64 descriptors/ring and a transposed gather needs n/16+2) are
independent of batch boundaries. Bulk spans reduce with tensor_reduce (plain
sum; the constant weight is applied once at the end), tail spans with
scalar_tensor_tensor + accum_out against the materialized tail weight.
Batch-parallel over 8 cores (4 batches each).
"""

import numpy as np
from contextlib import ExitStack

import concourse.bass as bass
import concourse.bacc as bacc
import concourse.tile as tile
from concourse import mybir

B, L, V, M, N = 32, 4096, 2000, 128, 64
LTAIL = 1024
NCORES = 8
BPC = B // NCORES
TOK = BPC * L
LN_EPS = 1e-5
VPAD = 2048          # vocab rows padded to 16 ranks of 128
NRANK = VPAD // 128

CHUNK = 896          # <= 992 (SWDGE ring: n/16+2 <= 64), multiple of 128

# prm column map ([128, 8] f32)
ALOG0, DCOL0, LOGDT0, WCLS0, BCLS0 = 0, 1, 2, 3, 5

f32 = mybir.dt.float32
bf16 = mybir.dt.bfloat16
i16 = mybir.dt.int16
AF = mybir.ActivationFunctionType
OP = mybir.AluOpType


def make_chunks(total, chunk):
    out, c0 = [], 0
    while c0 < total:
        n = min(chunk, total - c0)
        out.append((c0, n))
        c0 += n
    return out


def spans_for_chunk(c0, n):
    """Intersect [c0, c0+n) with each batch's bulk/tail ranges."""
    out = []
    for b in range(BPC):
        bs, be = b * L, b * L + (L - LTAIL)
        ts, te = b * L + (L - LTAIL), (b + 1) * L
        s0, s1 = max(c0, bs), min(c0 + n, be)
        if s0 < s1:
            out.append((s0, s1, b, False))
        s0, s1 = max(c0, ts), min(c0 + n, te)
        if s0 < s1:
            out.append((s0, s1, b, True))
    return out


CHUNKS = make_chunks(TOK, CHUNK)
NBULK = max(
    sum(1 for (c0, n) in CHUNKS for s in spans_for_chunk(c0, n)
        if s[2] == b and not s[3]) for b in range(BPC))
NTAIL = max(
    sum(1 for (c0, n) in CHUNKS for s in spans_for_chunk(c0, n)
        if s[2] == b and s[3]) for b in range(BPC))


def declare_io(nc):
    emb_d = nc.dram_tensor("emb_l", [128, NRANK * 128], f32, kind="ExternalInput")
    idx_d = nc.dram_tensor("idx_l", [16, TOK // 16], i16, kind="ExternalInput")
    prm_d = nc.dram_tensor("prm_l", [128, 8], f32, kind="ExternalInput")
    ct_d = nc.dram_tensor("ct_l", [64, 128], f32, kind="ExternalInput")
    lnwb_d = nc.dram_tensor("lnwb_l", [2, 128], f32, kind="ExternalInput")
    out_d = nc.dram_tensor("out", [BPC, 2], f32, kind="ExternalOutput")
    return emb_d, idx_d, prm_d, ct_d, lnwb_d, out_d


def emit_body(nc, tc, ctx, io, ln_affine):
    emb_d, idx_d, prm_d, ct_d, lnwb_d, out_d = io
    singles = ctx.enter_context(tc.tile_pool(name="singles", bufs=1))
    work = ctx.enter_context(tc.tile_pool(name="work", bufs=2))
    small = ctx.enter_context(tc.tile_pool(name="small", bufs=1))
    psum = ctx.enter_context(tc.tile_pool(name="psum", bufs=1, space="PSUM"))

    # ---- input loads. idx first (gates every gather), then emb quarters
    # (sync/SP ring); params + on-chip idx replication on the scalar/ACT ring.
    idx_sb = singles.tile([128, TOK // 16], i16)
    nc.sync.dma_start(out=idx_sb[0:16, :], in_=idx_d[:])
    emb_sb = singles.tile([128, NRANK, 128], f32)
    emb_v = emb_d[:].rearrange("p (r m) -> p r m", m=128)
    for hq in range(4):
        nc.sync.dma_start(out=emb_sb[:, 4 * hq:4 * hq + 4, :],
                          in_=emb_v[:, 4 * hq:4 * hq + 4, :])
    prm = singles.tile([128, 8], f32)
    nc.scalar.dma_start(out=prm, in_=prm_d[:])
    ct = singles.tile([64, 128], f32)
    nc.scalar.dma_start(out=ct, in_=ct_d[:])
    if ln_affine:
        lnwb = singles.tile([2, 128], f32)
        nc.scalar.dma_start(out=lnwb, in_=lnwb_d[:])
    nc.scalar.dma_start(out=idx_sb[16:32, :], in_=idx_sb[0:16, :])
    nc.scalar.dma_start(out=idx_sb[32:64, :], in_=idx_sb[0:32, :])
    nc.scalar.dma_start(out=idx_sb[64:128, :], in_=idx_sb[0:64, :])

    # ---- warmup gather: pays the SWDGE library/ring init under the loads
    wsrc = small.tile([128, 128], bf16)
    nc.vector.memset(wsrc, 0.0)
    widx = small.tile([128, 8], i16)
    nc.vector.memset(widx, 0)
    wout = small.tile([128, 1, 128], bf16)
    nc.gpsimd.dma_gather(
        out_ap=wout, in_ap=wsrc[:], idxs_ap=widx, num_idxs=128,
        num_idxs_reg=128, elem_size=128, transpose=True,
        sbuf_tokens_per_rank=128, sbuf_free_dim_per_rank=256, queue_num=0)

    # ---- S4D weight construction (tiny ops + 3 small matmuls)
    ramp = singles.tile([64, LTAIL], f32)   # LTAIL - j  (on-chip iota)
    nc.gpsimd.iota(ramp, pattern=[[-1, LTAIL]], base=LTAIL,
                   channel_multiplier=0, allow_small_or_imprecise_dtypes=True)
    dt_col = small.tile([128, 1], f32)
    nc.scalar.activation(dt_col, prm[:, LOGDT0:LOGDT0 + 1], AF.Exp)
    expa = small.tile([64, 1], f32)
    nc.scalar.activation(expa, prm[0:64, ALOG0:ALOG0 + 1], AF.Exp)
    ones_1x64 = small.tile([1, 64], f32)
    nc.vector.memset(ones_1x64, 1.0)
    dt0_ps = psum.tile([64, 1], f32)
    nc.tensor.matmul(dt0_ps, lhsT=ones_1x64, rhs=dt_col[0:1, :], start=True, stop=True)
    c_col = small.tile([64, 1], f32)  # c_n = -exp(A_log_n)*dt  (negative)
    nc.vector.scalar_tensor_tensor(
        out=c_col, in0=expa, scalar=-1.0, in1=dt0_ps, op0=OP.mult, op1=OP.mult)
    q_col = small.tile([64, 1], f32)
    nc.scalar.activation(q_col, c_col, AF.Exp)
    one_col = small.tile([64, 1], f32)
    nc.vector.memset(one_col, 1.0)
    omq = small.tile([64, 1], f32)
    nc.vector.tensor_sub(omq, one_col, q_col)
    wrec = small.tile([64, 1], f32)
    nc.vector.reciprocal(wrec, omq)
    g_sb = singles.tile([64, 128], f32)  # g^T[n,m] = C^T[n,m] / (1 - q_n)
    nc.vector.tensor_scalar_mul(g_sb, ct, scalar1=wrec)
    p_sb = singles.tile([64, LTAIL], f32)  # q_n^(LTAIL - j)
    nc.scalar.activation(p_sb, ramp, AF.Exp, scale=c_col)
    rho_ps = []
    for h in range(2):
        rp = psum.tile([128, 512], f32, tag=f"rho{h}")
        nc.tensor.matmul(rp, lhsT=g_sb, rhs=p_sb[:, h * 512:(h + 1) * 512],
                         start=True, stop=True)
        rho_ps.append(rp)
    kt_ps = psum.tile([128, 1], f32)
    nc.tensor.matmul(kt_ps, lhsT=g_sb, rhs=one_col, start=True, stop=True)
    wcd_raw = small.tile([128, 1], f32)  # Ktot + D (unscaled)
    nc.vector.tensor_add(wcd_raw, kt_ps, prm[:, DCOL0:DCOL0 + 1])
    wcd = small.tile([128, 1], f32)      # (Ktot + D) / L
    nc.vector.tensor_scalar_mul(wcd, wcd_raw, scalar1=1.0 / L)
    # tail weight W^T[m, j] = WcD_raw - rho[:, j], unscaled
    w_t = singles.tile([128, LTAIL], bf16)
    for h in range(2):
        sl = slice(h * 512, (h + 1) * 512)
        nc.vector.tensor_scalar(
            out=w_t[:, sl], in0=rho_ps[h], scalar1=-1.0, scalar2=wcd_raw,
            op0=OP.mult, op1=OP.add)

    # ---- partial-sum tiles for the fused reduce
    pcb = small.tile([128, BPC, NBULK], f32)
    nc.vector.memset(pcb, 0.0)
    pct = small.tile([128, BPC, NTAIL], f32)
    nc.vector.memset(pct, 0.0)

    # ---- LN of the vocab table, pipelined per emb quarter.
    # stats via sum + sum-of-squares (square on ACT); rsqrt via Quake seed +
    # 2 Newton steps on DVE (no sqrt ACT table -> one table load total).
    enorm = singles.tile([128, NRANK, 128], bf16)
    sume = small.tile([128, NRANK], f32)
    sumsq = small.tile([128, NRANK], f32)
    mean = small.tile([128, NRANK], f32)
    meansq = small.tile([128, NRANK], f32)
    vpe = small.tile([128, NRANK], f32)
    shi = small.tile([128, NRANK], mybir.dt.uint32)
    y = small.tile([128, NRANK], mybir.dt.int32)
    t1 = small.tile([128, NRANK], f32)
    rstd = small.tile([128, NRANK], f32)
    negmurs = small.tile([128, NRANK], f32)
    if ln_affine:
        ones_1x128 = small.tile([1, 128], f32)
        nc.vector.memset(ones_1x128, 1.0)
        lnw_ps = psum.tile([128, 128], f32, tag="lnw")
        nc.tensor.matmul(lnw_ps, lhsT=ones_1x128, rhs=lnwb[0:1, :], start=True, stop=True)
        lnb_ps = psum.tile([128, 128], f32, tag="lnb")
        nc.tensor.matmul(lnb_ps, lhsT=ones_1x128, rhs=lnwb[1:2, :], start=True, stop=True)
        lnw_bc = singles.tile([128, 128], f32)
        nc.scalar.copy(lnw_bc, lnw_ps)
        lnb_bc = singles.tile([128, 128], f32)
        nc.scalar.copy(lnb_bc, lnb_ps)
    for hq in range(4):
        qs = slice(4 * hq, 4 * hq + 4)
        nc.vector.tensor_reduce(out=sume[:, qs], in_=emb_sb[:, qs, :],
                                axis=mybir.AxisListType.X, op=OP.add)
        sqq = work.tile([128, 4, 128], f32, tag="sq")
        nc.scalar.square(sqq, emb_sb[:, qs, :])
        nc.vector.tensor_reduce(out=sumsq[:, qs], in_=sqq,
                                axis=mybir.AxisListType.X, op=OP.add)
        nc.vector.tensor_scalar_mul(mean[:, qs], sume[:, qs], scalar1=1.0 / 128.0)
        nc.vector.tensor_mul(meansq[:, qs], mean[:, qs], mean[:, qs])
        nc.vector.scalar_tensor_tensor(
            out=vpe[:, qs], in0=sumsq[:, qs], scalar=1.0 / 128.0, in1=meansq[:, qs],
            op0=OP.mult, op1=OP.subtract)
        nc.vector.tensor_single_scalar(vpe[:, qs], vpe[:, qs], LN_EPS, OP.add)
        nc.vector.tensor_single_scalar(
            shi[:, qs], vpe[:, qs].bitcast(mybir.dt.uint32), 1,
            OP.logical_shift_right)
        nc.vector.tensor_scalar(
            out=y[:, qs], in0=shi[:, qs], scalar1=-1.0,
            scalar2=float(0x5F3759DF), op0=OP.mult, op1=OP.add)
        cur = y[:, qs].bitcast(f32)
        for it in range(2):
            nc.vector.tensor_mul(t1[:, qs], cur, cur)
            nc.vector.scalar_tensor_tensor(
                out=t1[:, qs], in0=t1[:, qs], scalar=-0.5, in1=vpe[:, qs],
                op0=OP.mult, op1=OP.mult)
            nc.vector.scalar_tensor_tensor(
                out=rstd[:, qs], in0=t1[:, qs], scalar=1.5, in1=cur,
                op0=OP.add, op1=OP.mult)
            cur = rstd[:, qs]
        nc.vector.scalar_tensor_tensor(
            out=negmurs[:, qs], in0=mean[:, qs], scalar=-1.0,
            in1=rstd[:, qs], op0=OP.mult, op1=OP.mult)
        for r in range(4 * hq, 4 * hq + 4):
            if ln_affine:
                tmp = work.tile([128, 128], f32, tag="lntmp")
                nc.scalar.activation(tmp, emb_sb[:, r, :], AF.Identity,
                                     bias=negmurs[:, r:r + 1],
                                     scale=rstd[:, r:r + 1])
                nc.vector.tensor_mul(tmp, tmp, lnw_bc)
                nc.vector.tensor_add(enorm[:, r, :], tmp, lnb_bc)
            elif r % 2 == 0:
                nc.scalar.activation(enorm[:, r, :], emb_sb[:, r, :], AF.Identity,
                                     bias=negmurs[:, r:r + 1],
                                     scale=rstd[:, r:r + 1])
            else:
                nc.vector.tensor_scalar(
                    out=enorm[:, r, :], in0=emb_sb[:, r, :],
                    scalar1=mean[:, r:r + 1], scalar2=rstd[:, r:r + 1],
                    op0=OP.subtract, op1=OP.mult)

    # ---- chunked gather into one contiguous u tile + fused weighted reduce
    enorm_flat = enorm.rearrange("p r m -> p (r m)")
    u = singles.tile([128, TOK], bf16)
    kb = [0] * BPC
    kt = [0] * BPC
    for ci, (c0, n) in enumerate(CHUNKS):
        nc.gpsimd.dma_gather(
            out_ap=u[:, c0:c0 + n].rearrange("p (c l) -> p c l", c=1),
            in_ap=enorm_flat,
            idxs_ap=idx_sb[:, c0 // 16:(c0 + n) // 16],
            num_idxs=n,
            num_idxs_reg=n,
            elem_size=128,
            transpose=True,
            sbuf_tokens_per_rank=128,
            sbuf_free_dim_per_rank=256,
            queue_num=ci % 4,
        )
        for (s0, s1, b, is_tail) in spans_for_chunk(c0, n):
            if is_tail:
                off = s0 - (b * L + (L - LTAIL))
                prod = work.tile([128, CHUNK], bf16, tag="prod")
                nc.vector.scalar_tensor_tensor(
                    out=prod[:, :s1 - s0],
                    in0=u[:, s0:s1],
                    scalar=1.0 / L,
                    in1=w_t[:, off:off + (s1 - s0)],
                    op0=OP.mult,
                    op1=OP.mult,
                    accum_out=pct[:, b, kt[b]:kt[b] + 1],
                )
                kt[b] += 1
            else:
                nc.vector.tensor_reduce(
                    out=pcb[:, b, kb[b]:kb[b] + 1], in_=u[:, s0:s1],
                    axis=mybir.AxisListType.X, op=OP.add)
                kb[b] += 1

    # ---- combine partials: pooled[m,b] = wcd*sum_bulk + (1/L)*sum_tail
    bulkT = small.tile([128, BPC], f32)
    nc.vector.tensor_reduce(out=bulkT, in_=pcb, axis=mybir.AxisListType.X, op=OP.add)
    tailT = small.tile([128, BPC], f32)
    nc.vector.tensor_reduce(out=tailT, in_=pct, axis=mybir.AxisListType.X, op=OP.add)
    pooled = small.tile([128, BPC], f32)
    nc.vector.tensor_scalar_mul(pooled, bulkT, scalar1=wcd)
    nc.vector.tensor_add(pooled, pooled, tailT)

    # ---- classifier: logits = pooled^T @ Wcls^T + b_cls
    ones_1xb = small.tile([1, BPC], f32)
    nc.vector.memset(ones_1xb, 1.0)
    logits_ps = psum.tile([BPC, 2], f32)
    nc.tensor.matmul(logits_ps, lhsT=pooled, rhs=prm[:, WCLS0:WCLS0 + 2],
                     start=True, stop=False)
    nc.tensor.matmul(logits_ps, lhsT=ones_1xb, rhs=prm[0:1, BCLS0:BCLS0 + 2],
                     start=False, stop=True)
    out_sb = small.tile([BPC, 2], f32)
    nc.vector.tensor_copy(out_sb, logits_ps)
    nc.sync.dma_start(out=out_d[:], in_=out_sb)


def build_program(ln_affine: bool):
    nc = bacc.Bacc("TRN2", target_bir_lowering=False, debug=False,
                   num_swdge_queues=4)
    io = declare_io(nc)
    with tile.TileContext(nc) as tc:
        with ExitStack() as ctx:
            emit_body(nc, tc, ctx, io, ln_affine)
    nc.compile()
    return nc


_PROG_CACHE = {}


def _get_prog(ln_affine: bool):
    if ln_affine not in _PROG_CACHE:
        _PROG_CACHE[ln_affine] = build_program(ln_affine)
    return _PROG_CACHE[ln_affine]


def host_prep(x, emb, ln_w, ln_b, A_log, D, C_re, log_dt, W_cls, b_cls):
    """Pure data marshaling: reshape/transposes/pads, no arithmetic on values."""
    x = np.asarray(x)
    assert x.shape == (B, L)
    emb = np.asarray(emb, dtype=np.float32)
    assert np.allclose(log_dt, log_dt[0]), "factorized S4D path needs scalar dt"

    emb_pad = np.zeros((VPAD, M), dtype=np.float32)
    emb_pad[:V] = emb
    emb_l = np.ascontiguousarray(
        emb_pad.reshape(NRANK, 128, M).transpose(1, 0, 2).reshape(128, NRANK * M))

    prm = np.zeros((128, 8), dtype=np.float32)
    prm[0:64, ALOG0] = np.asarray(A_log, dtype=np.float32)
    prm[:, DCOL0] = np.asarray(D, dtype=np.float32)
    prm[:, LOGDT0] = np.asarray(log_dt, dtype=np.float32)
    prm[:, WCLS0:WCLS0 + 2] = np.asarray(W_cls, dtype=np.float32).T
    prm[0, BCLS0:BCLS0 + 2] = np.asarray(b_cls, dtype=np.float32)
    ct = np.ascontiguousarray(np.asarray(C_re, dtype=np.float32).T)
    lnwb = np.ascontiguousarray(
        np.stack([np.asarray(ln_w, dtype=np.float32),
                  np.asarray(ln_b, dtype=np.float32)]))

    ln_affine = not (np.all(np.asarray(ln_w) == 1.0) and np.all(np.asarray(ln_b) == 0.0))

    in_maps = []
    for k in range(NCORES):
        xc = x[k * BPC:(k + 1) * BPC].astype(np.int16).reshape(-1)   # [TOK]
        idx_l = np.ascontiguousarray(xc.reshape(-1, 16).T)           # [16, TOK/16]
        in_maps.append({"emb_l": emb_l, "idx_l": idx_l, "prm_l": prm,
                        "ct_l": ct, "lnwb_l": lnwb})
    return in_maps, ln_affine


def kernel(**inputs):
    from concourse.bass_utils import run_bass_kernel_spmd

    in_maps, ln_affine = host_prep(**inputs)
    nc = _get_prog(ln_affine)
    res = run_bass_kernel_spmd(nc, in_maps, core_ids=list(range(NCORES)))
    out = np.concatenate([res.results[k]["out"] for k in range(NCORES)], axis=0)
    return out.astype(np.float32)


# revision 20
# speedup vs baseline: 1.0769x; 1.0071x over previous
"""Trainium2 Bass kernel for nn_ExperimentModel (embed -> LN -> S4D -> mean-pool -> linear).

Math: the output is pooled over L, and mean-pooling commutes with the causal
conv, so  pooled[b,m] = (1/L) * sum_l u[b,l,m] * W[m,l]  with
  W[m,l] = CK[m, L-1-l] + D[m],   CK[m,t] = sum_n C[m,n] (1-q_n^{t+1})/(1-q_n),
  q_n = exp(dt * A_n)  (dt is a scalar: log_dt is uniform by construction).
W[m,l] = WcD[m] - rho[m, L-l],  rho[m,s] = sum_n g[m,n] q_n^s,  g = C/(1-q),
and rho is negligible for s > LTAIL (q_n <= e^-0.01): for l < L-LTAIL the
weight is the per-channel constant WcD[m].

u = LN(emb[x]) is a gather from a pre-normalized 2000-row bf16 table kept in
SBUF, fetched with SBUF-source transposed dma_gather into [m=128 part, l]
layout. Schedule (vs the first version): inputs are minimal (idx loaded once
into 16 partitions and replicated on-chip; the exp ramp is an on-chip iota;
params are three small tensors), the vocab-table LN is pipelined per emb
quarter via bn_stats/bn_aggr, a warmup dma_gather absorbs the SWDGE
ring/library init while the table is still loading, and all BPC*L positions
live in one contiguous u tile so the gather chunks (<=896 idxs: the SWDGE
ring holds 64 descriptors/ring and a transposed gather needs n/16+2) are
independent of batch boundaries. Bulk spans reduce with tensor_reduce (plain
sum; the constant weight is applied once at the end), tail spans with
scalar_tensor_tensor + accum_out against the materialized tail weight.
Batch-parallel over 8 cores (4 batches each).
"""

import numpy as np
from contextlib import ExitStack

import concourse.bass as bass
import concourse.bacc as bacc
import concourse.tile as tile
from concourse import mybir

B, L, V, M, N = 32, 4096, 2000, 128, 64
LTAIL = 1024
NCORES = 8
BPC = B // NCORES
TOK = BPC * L
LN_EPS = 1e-5
VPAD = 2048          # vocab rows padded to 16 ranks of 128
NRANK = VPAD // 128

CHUNK = 896          # <= 992 (SWDGE ring: n/16+2 <= 64), multiple of 128
NQ = 4               # SWDGE queues used round-robin (ucode MAX_SWDGE_QUEUES=4)

# prm column map ([128, 8] f32)
ALOG0, DCOL0, LOGDT0, WCLS0, BCLS0 = 0, 1, 2, 3, 5

PROBE = [False]      # timing probe: skip the per-chunk reduces

f32 = mybir.dt.float32
bf16 = mybir.dt.bfloat16
i16 = mybir.dt.int16
AF = mybir.ActivationFunctionType
OP = mybir.AluOpType


def make_chunks(total, chunk):
    out, c0 = [], 0
    while c0 < total:
        n = min(chunk, total - c0)
        out.append((c0, n))
        c0 += n
    return out


def spans_for_chunk(c0, n):
    """Intersect [c0, c0+n) with each batch's bulk/tail ranges."""
    out = []
    for b in range(BPC):
        bs, be = b * L, b * L + (L - LTAIL)
        ts, te = b * L + (L - LTAIL), (b + 1) * L
        s0, s1 = max(c0, bs), min(c0 + n, be)
        if s0 < s1:
            out.append((s0, s1, b, False))
        s0, s1 = max(c0, ts), min(c0 + n, te)
        if s0 < s1:
            out.append((s0, s1, b, True))
    return out


CHUNKS = make_chunks(TOK, CHUNK)
NBULK = max(
    sum(1 for (c0, n) in CHUNKS for s in spans_for_chunk(c0, n)
        if s[2] == b and not s[3]) for b in range(BPC))
NTAIL = max(
    sum(1 for (c0, n) in CHUNKS for s in spans_for_chunk(c0, n)
        if s[2] == b and s[3]) for b in range(BPC))


def declare_io(nc):
    emb_d = nc.dram_tensor("emb_l", [128, NRANK * 128], f32, kind="ExternalInput")
    idx_d = nc.dram_tensor("idx_l", [128, TOK // 16], i16, kind="ExternalInput")
    prm_d = nc.dram_tensor("prm_l", [128, 8], f32, kind="ExternalInput")
    ct_d = nc.dram_tensor("ct_l", [64, 128], f32, kind="ExternalInput")
    lnwb_d = nc.dram_tensor("lnwb_l", [2, 128], f32, kind="ExternalInput")
    out_d = nc.dram_tensor("out", [BPC, 2], f32, kind="ExternalOutput")
    return emb_d, idx_d, prm_d, ct_d, lnwb_d, out_d


def emit_body(nc, tc, ctx, io, ln_affine):
    emb_d, idx_d, prm_d, ct_d, lnwb_d, out_d = io
    singles = ctx.enter_context(tc.tile_pool(name="singles", bufs=1))
    work = ctx.enter_context(tc.tile_pool(name="work", bufs=2))
    small = ctx.enter_context(tc.tile_pool(name="small", bufs=1))
    psum = ctx.enter_context(tc.tile_pool(name="psum", bufs=1, space="PSUM"))

    # ---- input loads. idx first (gates every gather), then emb quarters
    # (sync/SP ring); params on the scalar/ACT ring.
    idx_sb = singles.tile([128, TOK // 16], i16)
    nc.sync.dma_start(out=idx_sb, in_=idx_d[:])
    emb_sb = singles.tile([128, NRANK, 128], f32)
    emb_v = emb_d[:].rearrange("p (r m) -> p r m", m=128)
    for hq in range(4):
        nc.sync.dma_start(out=emb_sb[:, 4 * hq:4 * hq + 4, :],
                          in_=emb_v[:, 4 * hq:4 * hq + 4, :])
    prm = singles.tile([128, 8], f32)
    nc.scalar.dma_start(out=prm, in_=prm_d[:])
    ct = singles.tile([64, 128], f32)
    nc.scalar.dma_start(out=ct, in_=ct_d[:])
    if ln_affine:
        lnwb = singles.tile([2, 128], f32)
        nc.scalar.dma_start(out=lnwb, in_=lnwb_d[:])

    # ---- warmup gather: pays the SWDGE library/ring init under the loads
    wsrc = small.tile([128, 128], bf16)
    nc.vector.memset(wsrc, 0.0)
    widx = small.tile([128, 8], i16)
    nc.vector.memset(widx, 0)
    wout = small.tile([128, 1, 128], bf16)
    with tc.high_priority():
        nc.gpsimd.dma_gather(
            out_ap=wout, in_ap=wsrc[:], idxs_ap=widx, num_idxs=128,
            num_idxs_reg=128, elem_size=128, transpose=True,
            sbuf_tokens_per_rank=128, sbuf_free_dim_per_rank=256, queue_num=3)

    # ---- S4D weight construction (tiny ops + 3 small matmuls)
    ramp = singles.tile([64, LTAIL], f32)   # LTAIL - j  (on-chip iota)
    nc.gpsimd.iota(ramp, pattern=[[-1, LTAIL]], base=LTAIL,
                   channel_multiplier=0, allow_small_or_imprecise_dtypes=True)
    dt_col = small.tile([128, 1], f32)
    nc.scalar.activation(dt_col, prm[:, LOGDT0:LOGDT0 + 1], AF.Exp)
    expa = small.tile([64, 1], f32)
    nc.scalar.activation(expa, prm[0:64, ALOG0:ALOG0 + 1], AF.Exp)
    ones_1x64 = small.tile([1, 64], f32)
    nc.vector.memset(ones_1x64, 1.0)
    dt0_ps = psum.tile([64, 1], f32)
    nc.tensor.matmul(dt0_ps, lhsT=ones_1x64, rhs=dt_col[0:1, :], start=True, stop=True)
    c_col = small.tile([64, 1], f32)  # c_n = -exp(A_log_n)*dt  (negative)
    nc.vector.scalar_tensor_tensor(
        out=c_col, in0=expa, scalar=-1.0, in1=dt0_ps, op0=OP.mult, op1=OP.mult)
    q_col = small.tile([64, 1], f32)
    nc.scalar.activation(q_col, c_col, AF.Exp)
    one_col = small.tile([64, 1], f32)
    nc.vector.memset(one_col, 1.0)
    omq = small.tile([64, 1], f32)
    nc.vector.tensor_sub(omq, one_col, q_col)
    wrec = small.tile([64, 1], f32)
    nc.vector.reciprocal(wrec, omq)
    g_sb = singles.tile([64, 128], f32)  # g^T[n,m] = C^T[n,m] / (1 - q_n)
    nc.vector.tensor_scalar_mul(g_sb, ct, scalar1=wrec)
    p_sb = singles.tile([64, LTAIL], f32)  # q_n^(LTAIL - j)
    nc.scalar.activation(p_sb, ramp, AF.Exp, scale=c_col)
    rho_ps = []
    for h in range(2):
        rp = psum.tile([128, 512], f32, tag=f"rho{h}")
        nc.tensor.matmul(rp, lhsT=g_sb, rhs=p_sb[:, h * 512:(h + 1) * 512],
                         start=True, stop=True)
        rho_ps.append(rp)
    kt_ps = psum.tile([128, 1], f32)
    nc.tensor.matmul(kt_ps, lhsT=g_sb, rhs=one_col, start=True, stop=True)
    wcd_raw = small.tile([128, 1], f32)  # Ktot + D (unscaled)
    nc.vector.tensor_add(wcd_raw, kt_ps, prm[:, DCOL0:DCOL0 + 1])
    wcd = small.tile([128, 1], f32)      # (Ktot + D) / L
    nc.vector.tensor_scalar_mul(wcd, wcd_raw, scalar1=1.0 / L)
    # tail weight W^T[m, j] = WcD_raw - rho[:, j], unscaled
    w_t = singles.tile([128, LTAIL], bf16)
    for h in range(2):
        sl = slice(h * 512, (h + 1) * 512)
        nc.vector.tensor_scalar(
            out=w_t[:, sl], in0=rho_ps[h], scalar1=-1.0, scalar2=wcd_raw,
            op0=OP.mult, op1=OP.add)

    # ---- partial-sum tiles for the fused reduce
    pcb = small.tile([128, BPC, NBULK], f32)
    nc.vector.memset(pcb, 0.0)
    pct = small.tile([128, BPC, NTAIL], f32)
    nc.vector.memset(pct, 0.0)

    # ---- LN of the vocab table, pipelined per emb quarter.
    # mean+var in one DVE pass per rank via bn_stats/bn_aggr; rsqrt via Quake
    # seed + 2 Newton steps on DVE (no sqrt ACT table -> one table load total).
    enorm = singles.tile([128, NRANK, 128], bf16)
    stats6 = small.tile([128, NRANK, 6], f32)
    msv = small.tile([128, NRANK, 2], f32)
    vpe = small.tile([128, NRANK], f32)
    shi = small.tile([128, NRANK], mybir.dt.uint32)
    y = small.tile([128, NRANK], mybir.dt.int32)
    t1 = small.tile([128, NRANK], f32)
    rstd = small.tile([128, NRANK], f32)
    negmurs = small.tile([128, NRANK], f32)
    if ln_affine:
        ones_1x128 = small.tile([1, 128], f32)
        nc.vector.memset(ones_1x128, 1.0)
        lnw_ps = psum.tile([128, 128], f32, tag="lnw")
        nc.tensor.matmul(lnw_ps, lhsT=ones_1x128, rhs=lnwb[0:1, :], start=True, stop=True)
        lnb_ps = psum.tile([128, 128], f32, tag="lnb")
        nc.tensor.matmul(lnb_ps, lhsT=ones_1x128, rhs=lnwb[1:2, :], start=True, stop=True)
        lnw_bc = singles.tile([128, 128], f32)
        nc.scalar.copy(lnw_bc, lnw_ps)
        lnb_bc = singles.tile([128, 128], f32)
        nc.scalar.copy(lnb_bc, lnb_ps)
    for hq in range(4):
        qs = slice(4 * hq, 4 * hq + 4)
        for r in range(4 * hq, 4 * hq + 4):
            nc.vector.bn_stats(stats6[:, r, :], emb_sb[:, r, :])
            nc.vector.bn_aggr(msv[:, r, :], stats6[:, r, :])
        nc.vector.tensor_single_scalar(vpe[:, qs], msv[:, qs, 1:2], LN_EPS, OP.add)
        nc.vector.tensor_single_scalar(
            shi[:, qs], vpe[:, qs].bitcast(mybir.dt.uint32), 1,
            OP.logical_shift_right)
        nc.vector.tensor_scalar(
            out=y[:, qs], in0=shi[:, qs], scalar1=-1.0,
            scalar2=float(0x5F3759DF), op0=OP.mult, op1=OP.add)
        cur = y[:, qs].bitcast(f32)
        for it in range(2):
            nc.vector.tensor_mul(t1[:, qs], cur, cur)
            nc.vector.scalar_tensor_tensor(
                out=t1[:, qs], in0=t1[:, qs], scalar=-0.5, in1=vpe[:, qs],
                op0=OP.mult, op1=OP.mult)
            nc.vector.scalar_tensor_tensor(
                out=rstd[:, qs], in0=t1[:, qs], scalar=1.5, in1=cur,
                op0=OP.add, op1=OP.mult)
            cur = rstd[:, qs]
        nc.vector.scalar_tensor_tensor(
            out=negmurs[:, qs], in0=msv[:, qs, 0:1], scalar=-1.0,
            in1=rstd[:, qs], op0=OP.mult, op1=OP.mult)
        for r in range(4 * hq, 4 * hq + 4):
            if ln_affine:
                tmp = work.tile([128, 128], f32, tag="lntmp")
                nc.scalar.activation(tmp, emb_sb[:, r, :], AF.Identity,
                                     bias=negmurs[:, r:r + 1],
                                     scale=rstd[:, r:r + 1])
                nc.vector.tensor_mul(tmp, tmp, lnw_bc)
                nc.vector.tensor_add(enorm[:, r, :], tmp, lnb_bc)
            elif r % 2 == 0:
                nc.scalar.activation(enorm[:, r, :], emb_sb[:, r, :], AF.Identity,
                                     bias=negmurs[:, r:r + 1],
                                     scale=rstd[:, r:r + 1])
            else:
                nc.vector.tensor_scalar(
                    out=enorm[:, r, :], in0=emb_sb[:, r, :],
                    scalar1=msv[:, r, 0:1], scalar2=rstd[:, r:r + 1],
                    op0=OP.subtract, op1=OP.mult)

    # ---- chunked gather into one contiguous u tile + fused weighted reduce
    enorm_flat = enorm.rearrange("p r m -> p (r m)")
    u = singles.tile([128, TOK], bf16)
    kb = [0] * BPC
    kt = [0] * BPC
    probe = PROBE[0]
    for ci, (c0, n) in enumerate(CHUNKS):
        nc.gpsimd.dma_gather(
            out_ap=u[:, c0:c0 + n].rearrange("p (c l) -> p c l", c=1),
            in_ap=enorm_flat,
            idxs_ap=idx_sb[:, c0 // 16:(c0 + n) // 16],
            num_idxs=n,
            num_idxs_reg=n,
            elem_size=128,
            transpose=True,
            sbuf_tokens_per_rank=128,
            sbuf_free_dim_per_rank=256,
            queue_num=ci % NQ,
        )
        for (s0, s1, b, is_tail) in spans_for_chunk(c0, n):
            if probe:
                continue
            if is_tail:
                off = s0 - (b * L + (L - LTAIL))
                prod = work.tile([128, CHUNK], bf16, tag="prod")
                nc.vector.scalar_tensor_tensor(
                    out=prod[:, :s1 - s0],
                    in0=u[:, s0:s1],
                    scalar=1.0 / L,
                    in1=w_t[:, off:off + (s1 - s0)],
                    op0=OP.mult,
                    op1=OP.mult,
                    accum_out=pct[:, b, kt[b]:kt[b] + 1],
                )
                kt[b] += 1
            else:
                nc.vector.tensor_reduce(
                    out=pcb[:, b, kb[b]:kb[b] + 1], in_=u[:, s0:s1],
                    axis=mybir.AxisListType.X, op=OP.add)
                kb[b] += 1

    if probe:
        # timing probe: one cheap consumer of u so the final drain covers it
        nc.vector.tensor_reduce(out=pcb[:, 0, 0:1], in_=u[:, 0:128],
                                axis=mybir.AxisListType.X, op=OP.add)
    # ---- combine partials: pooled[m,b] = wcd*sum_bulk + (1/L)*sum_tail
    bulkT = small.tile([128, BPC], f32)
    nc.vector.tensor_reduce(out=bulkT, in_=pcb, axis=mybir.AxisListType.X, op=OP.add)
    tailT = small.tile([128, BPC], f32)
    nc.vector.tensor_reduce(out=tailT, in_=pct, axis=mybir.AxisListType.X, op=OP.add)
    pooled = small.tile([128, BPC], f32)
    nc.vector.tensor_scalar_mul(pooled, bulkT, scalar1=wcd)
    nc.vector.tensor_add(pooled, pooled, tailT)

    # ---- classifier: logits = pooled^T @ Wcls^T + b_cls
    ones_1xb = small.tile([1, BPC], f32)
    nc.vector.memset(ones_1xb, 1.0)
    logits_ps = psum.tile([BPC, 2], f32)
    nc.tensor.matmul(logits_ps, lhsT=pooled, rhs=prm[:, WCLS0:WCLS0 + 2],
                     start=True, stop=False)
    nc.tensor.matmul(logits_ps, lhsT=ones_1xb, rhs=prm[0:1, BCLS0:BCLS0 + 2],
                     start=False, stop=True)
    out_sb = small.tile([BPC, 2], f32)
    nc.vector.tensor_copy(out_sb, logits_ps)
    nc.sync.dma_start(out=out_d[:], in_=out_sb)


def build_program(ln_affine: bool):
    nc = bacc.Bacc("TRN2", target_bir_lowering=False, debug=False,
                   num_swdge_queues=NQ)
    io = declare_io(nc)
    with tile.TileContext(nc) as tc:
        with ExitStack() as ctx:
            emit_body(nc, tc, ctx, io, ln_affine)
    nc.compile()
    return nc


_PROG_CACHE = {}


def _get_prog(ln_affine: bool):
    if ln_affine not in _PROG_CACHE:
        _PROG_CACHE[ln_affine] = build_program(ln_affine)
    return _PROG_CACHE[ln_affine]


def host_prep(x, emb, ln_w, ln_b, A_log, D, C_re, log_dt, W_cls, b_cls):
    """Pure data marshaling: reshape/transposes/pads, no arithmetic on values."""
    x = np.asarray(x)
    assert x.shape == (B, L)
    emb = np.asarray(emb, dtype=np.float32)
    assert np.allclose(log_dt, log_dt[0]), "factorized S4D path needs scalar dt"

    emb_pad = np.zeros((VPAD, M), dtype=np.float32)
    emb_pad[:V] = emb
    emb_l = np.ascontiguousarray(
        emb_pad.reshape(NRANK, 128, M).transpose(1, 0, 2).reshape(128, NRANK * M))

    prm = np.zeros((128, 8), dtype=np.float32)
    prm[0:64, ALOG0] = np.asarray(A_log, dtype=np.float32)
    prm[:, DCOL0] = np.asarray(D, dtype=np.float32)
    prm[:, LOGDT0] = np.asarray(log_dt, dtype=np.float32)
    prm[:, WCLS0:WCLS0 + 2] = np.asarray(W_cls, dtype=np.float32).T
    prm[0, BCLS0:BCLS0 + 2] = np.asarray(b_cls, dtype=np.float32)
    ct = np.ascontiguousarray(np.asarray(C_re, dtype=np.float32).T)
    lnwb = np.ascontiguousarray(
        np.stack([np.asarray(ln_w, dtype=np.float32),
                  np.asarray(ln_b, dtype=np.float32)]))

    ln_affine = not (np.all(np.asarray(ln_w) == 1.0) and np.all(np.asarray(ln_b) == 0.0))

    in_maps = []
    for k in range(NCORES):
        xc = x[k * BPC:(k + 1) * BPC].astype(np.int16).reshape(-1)   # [TOK]
        idx_l = np.ascontiguousarray(
            np.tile(xc.reshape(-1, 16).T, (8, 1)))                   # [128, TOK/16]
        in_maps.append({"emb_l": emb_l, "idx_l": idx_l, "prm_l": prm,
                        "ct_l": ct, "lnwb_l": lnwb})
    return in_maps, ln_affine


def kernel(**inputs):
    from concourse.bass_utils import run_bass_kernel_spmd

    in_maps, ln_affine = host_prep(**inputs)
    nc = _get_prog(ln_affine)
    res = run_bass_kernel_spmd(nc, in_maps, core_ids=list(range(NCORES)))
    out = np.concatenate([res.results[k]["out"] for k in range(NCORES)], axis=0)
    return out.astype(np.float32)
